# revision 1
# baseline (speedup 1.0000x reference)
"""Trainium2 Bass kernel for CognitionNetwork (GNN message passing + LSTM attention).

Contract: kernel(**inputs) takes FULL inputs, returns FULL [2048, 400] q_star.
Internally shards 2048 conversations (segments) contiguously across 8 NeuronCores
(256 segments each). Nodes are re-laid-out on the host so that each block of 32
segments owns a fixed number of 128-node tiles -> the SPMD device program is
fully static; all data-dependence lives in host-built indicator matrices.

Math notes:
  - segment softmax is computed WITHOUT max subtraction (scores are bounded,
    |e| < ~25 for this distribution; exp stays well inside f32 range).
  - softmax denominator comes for free from an appended ones-column on x.
  - LSTM bias is folded into the weight matrix via a ones-row on the input.
  - All matmuls run as float32r (full PE speed at N>=256, near-fp32 accuracy).
"""

import os
from contextlib import ExitStack

import numpy as np

import concourse.bass as bass
import concourse.bacc as bacc
import concourse.tile as tile
from concourse import masks, mybir
from concourse.bass_utils import run_bass_kernel_spmd

CORES = 8
B = 2048
F = 200
FW = F + 1            # x tile width: 200 features + ones column
SEG_PER_CORE = B // CORES   # 256
BS = 32               # segments per block
BLOCKS = SEG_PER_CORE // BS  # 8
STEPS = 3

TRACE = bool(int(os.environ.get("KERNEL_TRACE", "0")))
LAST_RESULT = None
_PROG_CACHE = {}


def _build_program(T_pad: int, nsteps: int = STEPS) -> bass.Bass:
    NT = BLOCKS * T_pad          # node tiles per core
    XW = NT * FW + 56            # packed x width (+56 so 256-wide reads never overrun)

    nc = bacc.Bacc("TRN2", target_bir_lowering=False, debug=False)
    f32 = mybir.dt.float32
    f32r = mybir.dt.float32r
    AF = mybir.ActivationFunctionType
    OP = mybir.AluOpType

    xt_d = nc.dram_tensor("xt", [128, XW], f32r, kind="ExternalInput").ap()
    w_d = nc.dram_tensor("w", [128, NT * BS], f32, kind="ExternalInput").ap()
    wt_d = nc.dram_tensor("wt", [128, 2 * T_pad * 128], f32r, kind="ExternalInput").ap()
    cosp_d = nc.dram_tensor("cosp", [128, NT], f32, kind="ExternalInput").ap()
    qs0t_d = nc.dram_tensor("qs0t", [401, 256], f32r, kind="ExternalInput").ap()
    ident_d = nc.dram_tensor("ident", [128, 128], f32r, kind="ExternalInput").ap()
    zro_d = nc.dram_tensor("zro", [128, 256], f32r, kind="ExternalInput").ap()
    wbig_d = nc.dram_tensor("wbig", [617, 800], f32r, kind="ExternalInput").ap()
    wc_d = nc.dram_tensor("wc", [424, 800], f32r, kind="ExternalInput").ap()
    qout_d = nc.dram_tensor("qout", [256, 400], f32, kind="ExternalOutput").ap()

    with tile.TileContext(nc) as tc:
        with ExitStack() as ctx:
            res = ctx.enter_context(tc.tile_pool(name="res", bufs=1))
            state = ctx.enter_context(tc.tile_pool(name="state", bufs=1))
            prodp = ctx.enter_context(tc.tile_pool(name="prodp", bufs=3))
            exwp = ctx.enter_context(tc.tile_pool(name="exwp", bufs=6))
            ebp = ctx.enter_context(tc.tile_pool(name="ebp", bufs=2))
            sbt = ctx.enter_context(tc.tile_pool(name="sbt", bufs=2))
            inp = ctx.enter_context(tc.tile_pool(name="inp", bufs=2))
            psA = ctx.enter_context(tc.tile_pool(name="psA", bufs=2, space="PSUM"))
            psB = ctx.enter_context(tc.tile_pool(name="psB", bufs=2, space="PSUM"))
            psC = ctx.enter_context(tc.tile_pool(name="psC", bufs=2, space="PSUM"))
            psT = ctx.enter_context(tc.tile_pool(name="psT", bufs=2, space="PSUM"))

            # ---------------- resident loads ----------------
            identity = res.tile([128, 128], f32r)
            nc.sync.dma_start(identity[:], ident_d[:])

            xt_sb = res.tile([128, XW], f32r)
            CW = T_pad * FW
            for g in range(BLOCKS):
                lo = g * CW
                hi = (g + 1) * CW if g < BLOCKS - 1 else XW
                nc.sync.dma_start(xt_sb[:, lo:hi], xt_d[:, lo:hi])

            w_sb = res.tile([128, NT * BS], f32)
            nc.sync.dma_start(w_sb[:], w_d[:])
            wt_sb = res.tile([128, 2 * T_pad * 128], f32r)
            nc.sync.dma_start(wt_sb[:], wt_d[:])
            cosp_sb = res.tile([128, NT], f32)
            nc.sync.dma_start(cosp_sb[:], cosp_d[:])

            # LSTM weights, step 1: [W_ih^T; zeros16; W_hh^T] rows + bias row
            wmat = []
            ksz = [128, 128, 128, 128, 104]
            koff = [0, 128, 256, 384, 512]
            for k, o in zip(ksz, koff):
                t = res.tile([k, 800], f32r, tag=f"wm{o}", name=f"wm{o}")
                nc.sync.dma_start(t[:], wbig_d[o : o + k, :])
                wmat.append(t)
            wbias = res.tile([1, 800], f32r)
            nc.sync.dma_start(wbias[:], wbig_d[616:617, :])
            ones_c = res.tile([1, 256], f32r)
            nc.sync.dma_start(ones_c[:], qs0t_d[400:401, :])

            # h, c, r state (seg-layout, two 128-partition halves)
            h_sb = [state.tile([128, 256], f32r, tag=f"h{i}", name=f"h{i}") for i in range(2)]
            c_sb = [state.tile([128, F], f32, tag=f"c{i}", name=f"c{i}") for i in range(2)]
            r_sb = [state.tile([128, F], f32r, tag=f"r{i}", name=f"r{i}") for i in range(2)]
            for i in range(2):
                nc.sync.dma_start(h_sb[i][:], zro_d[:])
                nc.vector.memset(c_sb[i][:], 0.0)

            # ---------------- phase 0: h0 = segment_sum(cos * x) ----------------
            for g in range(BLOCKS):
                h0ps = psB.tile([32, 256], f32, tag="rblk")
                for i in range(T_pad):
                    t = g * T_pad + i
                    cw = exwp.tile([128, BS], f32r, tag="exw")
                    nc.vector.tensor_scalar_mul(
                        cw[:], w_sb[:, t * BS : (t + 1) * BS], cosp_sb[:, t : t + 1]
                    )
                    nc.tensor.matmul(
                        h0ps[:],
                        lhsT=cw[:],
                        rhs=xt_sb[:, t * FW : t * FW + 256],
                        start=(i == 0),
                        stop=(i == T_pad - 1),
                    )
                dst = h_sb[g // 4]
                p0 = 32 * (g % 4)
                nc.vector.tensor_copy(dst[p0 : p0 + 32, 0:F], h0ps[:, 0:F])

            # ---------------- steps ----------------
            for s in range(nsteps):
                # ---- build transposed LSTM input chunks ----
                # SBUF compute APs must start at partition 0/32/64/96 (with
                # counts <=128/32/64/32), so chunk sections sit at 32-aligned
                # offsets with zero padding; PSUM sources are unrestricted.
                if s == 0:
                    # input rows = [q_star0 (400); zeros (16); h0 (200)] + ones
                    # A3 = [qs0[384:400]; pad16; h0T rows 0:96]
                    # A4 = [h0T rows 96:200]  (104 rows)
                    A0 = inp.tile([128, 256], f32r, tag="B0")
                    A1 = inp.tile([128, 256], f32r, tag="B1")
                    A2 = inp.tile([128, 256], f32r, tag="B2")
                    A3 = inp.tile([128, 256], f32r, tag="A3")
                    A4 = inp.tile([104, 256], f32r, tag="A4")
                    nc.sync.dma_start(A3[0:32, :], zro_d[0:32, :])
                    nc.sync.dma_start(A0[:], qs0t_d[0:128, :])
                    nc.sync.dma_start(A1[:], qs0t_d[128:256, :])
                    nc.sync.dma_start(A2[:], qs0t_d[256:384, :])
                    nc.sync.dma_start(A3[0:16, :], qs0t_d[384:400, :])
                    for half in range(2):
                        src = h_sb[half]
                        co = 128 * half
                        th = psT.tile([128, 128], f32r, tag="tp")
                        nc.tensor.transpose(th[:], src[:, 0:128], identity[:])
                        nc.vector.tensor_copy(A3[32:64, co : co + 128], th[0:32, :])
                        nc.vector.tensor_copy(A3[64:96, co : co + 128], th[32:64, :])
                        nc.vector.tensor_copy(A3[96:128, co : co + 128], th[64:96, :])
                        nc.vector.tensor_copy(A4[0:32, co : co + 128], th[96:128, :])
                        tl = psT.tile([72, 128], f32r, tag="tp")
                        nc.tensor.transpose(tl[:], src[:, 128:200], identity[:])
                        nc.vector.tensor_copy(A4[32:64, co : co + 128], tl[0:32, :])
                        nc.vector.tensor_copy(A4[64:96, co : co + 128], tl[32:64, :])
                        nc.vector.tensor_copy(A4[96:104, co : co + 128], tl[64:72, :])
                    chunks = [(A0, 128), (A1, 128), (A2, 128), (A3, 128), (A4, 104), (ones_c, 1)]
                    wtiles = wmat + [wbias]
                else:
                    # input rows = [h (200); zeros (24); r (200)] + ones
                    # C1 = [hT 128:200; pad24; rT 0:32], C2 = rT 32:160, C3 = rT 160:200
                    B0 = inp.tile([128, 256], f32r, tag="B0")
                    B1 = inp.tile([128, 256], f32r, tag="B1")
                    B2 = inp.tile([128, 256], f32r, tag="B2")
                    B3 = inp.tile([40, 256], f32r, tag="B3")
                    nc.sync.dma_start(B1[64:96, :], zro_d[0:32, :])
                    for half in range(2):
                        hs = h_sb[half]
                        rs = r_sb[half]
                        co = 128 * half
                        th = psT.tile([128, 128], f32r, tag="tp")
                        nc.tensor.transpose(th[:], hs[:, 0:128], identity[:])
                        nc.vector.tensor_copy(B0[:, co : co + 128], th[:])
                        tl = psT.tile([72, 128], f32r, tag="tp")
                        nc.tensor.transpose(tl[:], hs[:, 128:200], identity[:])
                        nc.vector.tensor_copy(B1[0:72, co : co + 128], tl[:])
                        tr = psT.tile([128, 128], f32r, tag="tp")
                        nc.tensor.transpose(tr[:], rs[:, 0:128], identity[:])
                        nc.vector.tensor_copy(B1[96:128, co : co + 128], tr[0:32, :])
                        nc.vector.tensor_copy(B2[0:32, co : co + 128], tr[32:64, :])
                        nc.vector.tensor_copy(B2[32:64, co : co + 128], tr[64:96, :])
                        nc.vector.tensor_copy(B2[64:96, co : co + 128], tr[96:128, :])
                        tq = psT.tile([72, 128], f32r, tag="tp")
                        nc.tensor.transpose(tq[:], rs[:, 128:200], identity[:])
                        nc.vector.tensor_copy(B2[96:128, co : co + 128], tq[0:32, :])
                        nc.vector.tensor_copy(B3[0:32, co : co + 128], tq[32:64, :])
                        nc.vector.tensor_copy(B3[32:40, co : co + 128], tq[64:72, :])
                    chunks = [(B0, 128), (B1, 128), (B2, 128), (B3, 40), (ones_c, 1)]
                    wtiles = wmat[:4] + [wbias]  # wmat reloaded with wc after step 1

                # ---- gates + cell update, per 128-segment half ----
                for half in range(2):
                    co = 128 * half
                    acts = {}
                    for part in range(2):  # part 0 -> i|f gates, part 1 -> g|o
                        ps = psC.tile([128, 400], f32, tag="gates")
                        nch = len(chunks)
                        for ci, (ctile, kdim) in enumerate(chunks):
                            nc.tensor.matmul(
                                ps[:],
                                lhsT=ctile[0:kdim, co : co + 128],
                                rhs=wtiles[ci][0:kdim, 400 * part : 400 * part + 400],
                                start=(ci == 0),
                                stop=(ci == nch - 1),
                            )
                        if part == 0:
                            si = sbt.tile([128, F], f32, tag="si")
                            nc.scalar.activation(si[:], ps[:, 0:F], AF.Sigmoid)
                            sf = sbt.tile([128, F], f32, tag="sf")
                            nc.scalar.activation(sf[:], ps[:, F:400], AF.Sigmoid)
                            acts["i"], acts["f"] = si, sf
                        else:
                            tg = sbt.tile([128, F], f32, tag="tg")
                            nc.scalar.activation(tg[:], ps[:, 0:F], AF.Tanh)
                            so = sbt.tile([128, F], f32, tag="so")
                            nc.scalar.activation(so[:], ps[:, F:400], AF.Sigmoid)
                            acts["g"], acts["o"] = tg, so
                    # c = sigm(f)*c + sigm(i)*tanh(g);  h = sigm(o)*tanh(c)
                    ch = c_sb[half]
                    tmp = sbt.tile([128, F], f32, tag="tmp")
                    nc.vector.tensor_mul(tmp[:], acts["f"][:], ch[:])
                    nc.vector.tensor_mul(ch[:], acts["i"][:], acts["g"][:])
                    nc.vector.tensor_add(ch[:], tmp[:], ch[:])
                    tct = sbt.tile([128, F], f32, tag="tct")
                    nc.scalar.activation(tct[:], ch[:], AF.Tanh)
                    nc.vector.tensor_mul(h_sb[half][:, 0:F], acts["o"][:], tct[:])

                # step 1 only: swap in the combined weights for steps 2..3
                if s == 0:
                    for k, o, t in zip([128, 128, 128, 40], [0, 128, 256, 384], wmat[:4]):
                        nc.sync.dma_start(t[0:k, :], wc_d[o : o + k, :])

                # ---- attention: e, softmax, r ----
                for g in range(BLOCKS):
                    lane = g % 4
                    p0 = 32 * lane
                    qt = h_sb[g // 4]
                    eb = ebp.tile([128, T_pad], f32, tag="eb")
                    exb = ebp.tile([128, T_pad], f32, tag="exb")
                    for i in range(T_pad):
                        t = g * T_pad + i
                        slot = (g // 4) * T_pad + i
                        qg = psA.tile([128, 256], f32, tag="qg")
                        nc.tensor.matmul(
                            qg[:],
                            lhsT=wt_sb[p0 : p0 + 32, 128 * slot : 128 * slot + 128],
                            rhs=qt[p0 : p0 + 32, 0:256],
                            start=True,
                            stop=True,
                            tile_position=(p0, 0),
                        )
                        prod = prodp.tile([128, F], f32, tag="prod")
                        nc.vector.scalar_tensor_tensor(
                            out=prod[:],
                            in0=xt_sb[:, t * FW : t * FW + F].bitcast(f32),
                            scalar=1.0,
                            in1=qg[:, 0:F],
                            op0=OP.mult,
                            op1=OP.mult,
                            accum_out=eb[:, i : i + 1],
                        )
                    nc.scalar.activation(exb[:], eb[:], AF.Exp)
                    rps = psB.tile([32, 256], f32, tag="rblk")
                    for i in range(T_pad):
                        t = g * T_pad + i
                        exw = exwp.tile([128, BS], f32r, tag="exw")
                        nc.vector.tensor_scalar_mul(
                            exw[:], w_sb[:, t * BS : (t + 1) * BS], exb[:, i : i + 1]
                        )
                        nc.tensor.matmul(
                            rps[:],
                            lhsT=exw[:],
                            rhs=xt_sb[:, t * FW : t * FW + 256],
                            start=(i == 0),
                            stop=(i == T_pad - 1),
                        )
                    dinv = sbt.tile([32, 1], f32, tag="dinv")
                    nc.vector.reciprocal(dinv[:], rps[:, F : F + 1])
                    rdst = r_sb[g // 4]
                    nc.vector.tensor_scalar_mul(
                        rdst[p0 : p0 + 32, 0:F], rps[:, 0:F], dinv[:]
                    )

            # ---------------- output: q_star = [h | r] ----------------
            for half in range(2):
                ro = 128 * half
                nc.sync.dma_start(qout_d[ro : ro + 128, 0:F], h_sb[half][:, 0:F].bitcast(f32))
                if nsteps > 0:
                    nc.sync.dma_start(qout_d[ro : ro + 128, F : 2 * F], r_sb[half][:, 0:F].bitcast(f32))

    nc.compile()
    return nc


def _get_program(T_pad: int) -> bass.Bass:
    nsteps = int(os.environ.get("KERNEL_NSTEPS", str(STEPS)))
    key = (T_pad, nsteps)
    if key not in _PROG_CACHE:
        _PROG_CACHE[key] = _build_program(T_pad, nsteps)
    return _PROG_CACHE[key]


def make_in_maps(x, batch, cos_coef, q_star, W_ih, W_hh, b_ih, b_hh):
    """Host-side shard + re-layout. Returns (in_maps, T_pad)."""
    x = np.ascontiguousarray(np.asarray(x, dtype=np.float32))
    batch = np.asarray(batch).astype(np.int64)
    cos = np.asarray(cos_coef, dtype=np.float32)
    qs = np.asarray(q_star, dtype=np.float32)
    W_ih = np.asarray(W_ih, dtype=np.float32)
    W_hh = np.asarray(W_hh, dtype=np.float32)
    bsum = (np.asarray(b_ih, dtype=np.float32) + np.asarray(b_hh, dtype=np.float32))

    counts = np.bincount(batch, minlength=B)
    starts = np.zeros(B + 1, dtype=np.int64)
    starts[1:] = np.cumsum(counts)
    blk_counts = counts.reshape(-1, BS).sum(axis=1)
    T_pad = int(max(1, -(-blk_counts.max() // 128)))
    NT = BLOCKS * T_pad
    XW = NT * FW + 56

    z16 = np.zeros((16, 800), dtype=np.float32)
    z24 = np.zeros((24, 800), dtype=np.float32)
    wbig = np.concatenate(
        [W_ih.T, z16, W_hh.T, bsum[None, :]], axis=0
    ).astype(np.float32)  # [617, 800]
    wc = np.concatenate(
        [W_ih[:, :F].T + W_hh.T, z24, W_ih[:, F:].T], axis=0
    ).astype(np.float32)  # [424, 800]

    in_maps = []
    for c in range(CORES):
        seg0 = c * SEG_PER_CORE
        xt = np.zeros((128, XW), dtype=np.float32)
        w = np.zeros((128, NT * BS), dtype=np.float32)
        wt = np.zeros((128, 2 * T_pad * 128), dtype=np.float32)
        cosp = np.zeros((128, NT), dtype=np.float32)
        for g in range(BLOCKS):
            sa = seg0 + g * BS
            n0, n1 = int(starts[sa]), int(starts[sa + BS])
            cnt = n1 - n0
            js = (batch[n0:n1] - sa).astype(np.int64)

            xb = np.zeros((T_pad * 128, FW), dtype=np.float32)
            xb[:cnt, :F] = x[n0:n1]
            xb[:cnt, F] = 1.0
            xt[:, g * T_pad * FW : (g + 1) * T_pad * FW] = (
                xb.reshape(T_pad, 128, FW).transpose(1, 0, 2).reshape(128, T_pad * FW)
            )

            wb = np.zeros((T_pad * 128, BS), dtype=np.float32)
            wb[np.arange(cnt), js] = 1.0
            w[:, g * T_pad * BS : (g + 1) * T_pad * BS] = (
                wb.reshape(T_pad, 128, BS).transpose(1, 0, 2).reshape(128, T_pad * BS)
            )

            cb = np.zeros(T_pad * 128, dtype=np.float32)
            cb[:cnt] = cos[n0:n1]
            cosp[:, g * T_pad : (g + 1) * T_pad] = cb.reshape(T_pad, 128).T

            wb3 = wb.reshape(T_pad, 128, BS)
            lane = g % 4
            for i in range(T_pad):
                slot = (g // 4) * T_pad + i
                wt[32 * lane : 32 * lane + 32, 128 * slot : 128 * slot + 128] = wb3[i].T

        qs0t = np.ones((401, 256), dtype=np.float32)
        qs0t[0:400] = qs[seg0 : seg0 + SEG_PER_CORE].T
        in_maps.append(
            {
                "xt": xt,
                "w": w,
                "wt": wt,
                "cosp": cosp,
                "qs0t": qs0t,
                "ident": np.eye(128, dtype=np.float32),
                "zro": np.zeros((128, 256), dtype=np.float32),
                "wbig": wbig,
                "wc": wc,
            }
        )
    return in_maps, T_pad


def kernel(x, batch, cos_coef, q_star, W_ih, W_hh, b_ih, b_hh):
    global LAST_RESULT
    in_maps, T_pad = make_in_maps(
        x, batch, cos_coef, q_star, W_ih, W_hh, b_ih, b_hh
    )
    nc = _get_program(T_pad)
    res = run_bass_kernel_spmd(nc, in_maps, list(range(CORES)), trace=TRACE)
    LAST_RESULT = res
    out = np.zeros((B, 2 * F), dtype=np.float32)
    for c in range(CORES):
        out[c * SEG_PER_CORE : (c + 1) * SEG_PER_CORE] = res.results[c]["qout"]
    return out



# revision 9
# speedup vs baseline: 1.7832x; 1.7832x over previous
"""Trainium2 Bass kernel for CognitionNetwork (GNN message passing + LSTM attention).

Contract: kernel(**inputs) takes FULL inputs, returns FULL [2048, 400] q_star.
Shards 2048 conversations contiguously across 8 NeuronCores (256 segments each);
each block of 32 segments owns T_pad 128-node tiles (host re-layout).

v2 design (vs v0 per-tile gather):
  - attention scores e come from block-level matmuls contracting FEATURES:
    weights = per-block Q^T (reused across the block's tiles), rhs = a
    feature-major fp16 copy of x. The segment mask is folded into 33 extra
    "features" (indicator rows * 100 on both sides, ones row * -100), so
    e_aug = e + 100*onehot - 100 and exp(e_aug) is already the masked,
    unnormalized attention weight (off-segment entries underflow to 0).
  - exp runs on the scalar engine straight out of PSUM into a bf16 tile;
    per-tile PE transposes flip it node-major; the r matmul streams a bf16
    node-major x copy (ones column appended -> denominator for free).
  - all matmul operands are 16-bit (fp16 for e/LSTM, bf16 for r/phase0):
    1 cycle/row at any output width; fp32 masters kept for h/c/r state.
"""

import os
from contextlib import ExitStack

import ml_dtypes
import numpy as np

import concourse.bass as bass
import concourse.bacc as bacc
import concourse.tile as tile
from concourse import mybir
from concourse.bass_utils import run_bass_kernel_spmd

CORES = 8
B = 2048
F = 200
FW = 202              # node-major x tile width: 200 feats + ones col + pad
SEG_PER_CORE = B // CORES   # 256
BS = 32               # segments per block
BLOCKS = SEG_PER_CORE // BS  # 8
STEPS = 3
KAUG = F + BS + 1     # 233 feature rows incl mask aug
K2 = KAUG - 128       # 105 rows in chunk 2

TRACE = bool(int(os.environ.get("KERNEL_TRACE", "0")))
LAST_RESULT = None
_PROG_CACHE = {}


def _build_program(T_pad: int, nsteps: int = STEPS) -> bass.Bass:
    NT = BLOCKS * T_pad          # node tiles per core
    XFW = NT * 128               # feature-major x width (nodes)
    BW = T_pad * 128             # nodes per block

    nc = bacc.Bacc("TRN2", target_bir_lowering=False, debug=False)
    f32 = mybir.dt.float32
    f32r = mybir.dt.float32r
    f16 = mybir.dt.float16
    bf16 = mybir.dt.bfloat16
    AF = mybir.ActivationFunctionType

    xnm_d = nc.dram_tensor("xnm", [128, NT * FW], bf16, kind="ExternalInput").ap()
    xf1_d = nc.dram_tensor("xf1", [128, XFW], f16, kind="ExternalInput").ap()
    xf2_d = nc.dram_tensor("xf2", [K2, XFW], f16, kind="ExternalInput").ap()
    cwt_d = nc.dram_tensor("cwt", [128, NT * BS], f16, kind="ExternalInput").ap()
    xp_d = nc.dram_tensor("xp", [128, NT * F], f16, kind="ExternalInput").ap()
    qs0t_d = nc.dram_tensor("qs0t", [401, 256], f16, kind="ExternalInput").ap()
    w0_d = nc.dram_tensor("w0", [634, 800], f16, kind="ExternalInput").ap()
    wc_d = nc.dram_tensor("wc", [434, 800], f16, kind="ExternalInput").ap()
    qc2c_d = nc.dram_tensor("qc2c", [BS + 1, 256], f16, kind="ExternalInput").ap()
    ones_d = nc.dram_tensor("onesr", [1, 256], f16, kind="ExternalInput").ap()
    idf_d = nc.dram_tensor("idf", [128, 128], f32r, kind="ExternalInput").ap()
    idb_d = nc.dram_tensor("idb", [BS, BS], bf16, kind="ExternalInput").ap()
    qout_d = nc.dram_tensor("qout", [256, 400], f32, kind="ExternalOutput").ap()

    with tile.TileContext(nc) as tc:
        with ExitStack() as ctx:
            res = ctx.enter_context(tc.tile_pool(name="res", bufs=1))
            state = ctx.enter_context(tc.tile_pool(name="state", bufs=1))
            eap = ctx.enter_context(tc.tile_pool(name="eap", bufs=2))
            xpp = ctx.enter_context(tc.tile_pool(name="xpp", bufs=3))
            eanp = ctx.enter_context(tc.tile_pool(name="eanp", bufs=2))
            sbt = ctx.enter_context(tc.tile_pool(name="sbt", bufs=2))
            psE = ctx.enter_context(tc.tile_pool(name="psE", bufs=2, space="PSUM"))
            psG = ctx.enter_context(tc.tile_pool(name="psG", bufs=2, space="PSUM"))
            psT = ctx.enter_context(tc.tile_pool(name="psT", bufs=2, space="PSUM"))
            psR = ctx.enter_context(tc.tile_pool(name="psR", bufs=2, space="PSUM"))

            # ---------------- resident loads ----------------
            idf = res.tile([128, 128], f32r)
            nc.sync.dma_start(idf[:], idf_d[:])
            idb = res.tile([BS, BS], bf16)
            nc.sync.dma_start(idb[:], idb_d[:])

            cwt_sb = res.tile([128, NT * BS], f16)
            nc.sync.dma_start(cwt_sb[:], cwt_d[:])
            xnm_sb = res.tile([128, NT * FW], bf16)
            for g in range(BLOCKS):
                nc.sync.dma_start(
                    xnm_sb[:, g * T_pad * FW : (g + 1) * T_pad * FW],
                    xnm_d[:, g * T_pad * FW : (g + 1) * T_pad * FW],
                )
            xf1_sb = res.tile([128, XFW], f16)
            xf2_sb = res.tile([K2, XFW], f16)
            for g in range(BLOCKS):
                nc.sync.dma_start(xf1_sb[:, g * BW : (g + 1) * BW], xf1_d[:, g * BW : (g + 1) * BW])
                nc.sync.dma_start(xf2_sb[:, g * BW : (g + 1) * BW], xf2_d[:, g * BW : (g + 1) * BW])

            # LSTM weights: step0 chunks E0..E3,F0,F1 ; steps>=1 chunks D0..D3
            wE = []
            for k, o in zip([128, 128, 128, 17, 128, K2], [0, 128, 256, 384, 401, 529]):
                t = res.tile([k, 800], f16, tag=f"wE{o}", name=f"wE{o}")
                nc.sync.dma_start(t[:], w0_d[o : o + k, :])
                wE.append(t)
            wD = []
            for k, o in zip([128, K2, 128, 73], [0, 128, 233, 361]):
                t = res.tile([k, 800], f16, tag=f"wD{o}", name=f"wD{o}")
                nc.sync.dma_start(t[:], wc_d[o : o + k, :])
                wD.append(t)

            # step-0 LSTM input chunks (q_star0^T from host)
            qsE = []
            for k, o in zip([128, 128, 128, 17], [0, 128, 256, 384]):
                t = res.tile([k, 256], f16, tag=f"qsE{o}", name=f"qsE{o}")
                nc.sync.dma_start(t[:], qs0t_d[o : o + k, :])
                qsE.append(t)

            # transposed-input chunks: Q1/Q2 (h^T + mask const), R1/R2 (r^T + ones)
            Q1 = res.tile([128, 256], f16, tag="Q1", name="Q1")
            Q2 = res.tile([K2, 256], f16, tag="Q2", name="Q2")
            nc.sync.dma_start(Q2[72:K2, :], qc2c_d[:])
            R1 = res.tile([128, 256], f16, tag="R1", name="R1")
            R2 = res.tile([73, 256], f16, tag="R2", name="R2")
            nc.sync.dma_start(R2[72:73, :], ones_d[:])

            # fp32 state masters (seg-major, two 128-partition halves)
            h_sb = [state.tile([128, F], f32r, tag=f"h{i}", name=f"h{i}") for i in range(2)]
            c_sb = [state.tile([128, F], f32, tag=f"c{i}", name=f"c{i}") for i in range(2)]
            r_sb = [state.tile([128, F], f32r, tag=f"r{i}", name=f"r{i}") for i in range(2)]
            for i in range(2):
                nc.vector.memset(c_sb[i][:], 0.0)

            # ---------------- phase 0: h0 = segment_sum(cos * x) ----------------
            for g in range(BLOCKS):
                xp = xpp.tile([128, T_pad * F], f16, tag="xp")
                nc.sync.dma_start(xp[:], xp_d[:, g * T_pad * F : (g + 1) * T_pad * F])
                h0ps = psR.tile([32, F], f32, tag="rblk")
                for i in range(T_pad):
                    t = g * T_pad + i
                    nc.tensor.matmul(
                        h0ps[:],
                        lhsT=cwt_sb[:, t * BS : (t + 1) * BS],
                        rhs=xp[:, i * F : (i + 1) * F],
                        start=(i == 0),
                        stop=(i == T_pad - 1),
                    )
                dst = h_sb[g // 4]
                p0 = 32 * (g % 4)
                nc.vector.tensor_copy(dst[p0 : p0 + 32, 0:F], h0ps[:])

            def emit_hT(src_halves, dst1, dst2):
                """transpose seg-major [128,200] f32 halves into fp16 feat-major
                chunks: dst1[:, co:co+128] rows 0..127, dst2[0:72, ...] rows 128..199."""
                for half in range(2):
                    src = src_halves[half]
                    co = 128 * half
                    t1 = psT.tile([128, 128], f32r, tag="tp")
                    nc.tensor.transpose(t1[:], src[:, 0:128], idf[:])
                    nc.vector.tensor_copy(dst1[:, co : co + 128], t1[:].bitcast(f32))
                    t2 = psT.tile([72, 128], f32r, tag="tp")
                    nc.tensor.transpose(t2[:], src[:, 128:200], idf[:])
                    nc.vector.tensor_copy(dst2[0:72, co : co + 128], t2[:].bitcast(f32))

            emit_hT(h_sb, Q1, Q2)

            # ---------------- steps ----------------
            NCH = (BW + 511) // 512  # 512-col e-matmul chunks per block

            def emit_e(g):
                """e_aug matmuls + exp -> EA (seg-major bf16 masked attention)."""
                ea = eap.tile([32, BW], bf16, tag="ea", name=f"ea")
                for k in range(NCH):
                    c0 = k * 512
                    cw = min(512, BW - c0)
                    pe = psE.tile([32, 512], f32, tag="pe")
                    nc.tensor.matmul(
                        pe[:, 0:cw],
                        lhsT=Q1[:, BS * g : BS * (g + 1)],
                        rhs=xf1_sb[:, g * BW + c0 : g * BW + c0 + cw],
                        start=True,
                        stop=False,
                    )
                    nc.tensor.matmul(
                        pe[:, 0:cw],
                        lhsT=Q2[0:K2, BS * g : BS * (g + 1)],
                        rhs=xf2_sb[0:K2, g * BW + c0 : g * BW + c0 + cw],
                        start=False,
                        stop=True,
                    )
                    nc.scalar.activation(ea[:, c0 : c0 + cw], pe[:, 0:cw], AF.Exp)
                return ea

            def emit_attn_tail(g, ea):
                """transpose EA node-major, r matmuls, normalize into r_sb."""
                rps = psR.tile([32, F + 1], f32, tag="rblk")
                NG = (T_pad + 3) // 4
                ean_prev = None
                for k in range(NG):
                    n4 = min(4, T_pad - 4 * k)
                    tp = psT.tile([128, 128], bf16, tag="tp")
                    for i4 in range(n4):
                        i = 4 * k + i4
                        nc.tensor.transpose(
                            tp[:, 32 * i4 : 32 * i4 + 32],
                            ea[:, 128 * i : 128 * i + 128],
                            idb[:],
                        )
                    ean = eanp.tile([128, 128], bf16, tag="ean")
                    nc.vector.tensor_copy(ean[:, 0 : 32 * n4], tp[:, 0 : 32 * n4])
                    if ean_prev is not None:
                        _emit_r(g, k - 1, 4, ean_prev, rps)
                    ean_prev = ean
                _emit_r(g, NG - 1, T_pad - 4 * (NG - 1), ean_prev, rps)
                dinv = sbt.tile([32, 1], f32, tag="dinv")
                nc.vector.reciprocal(dinv[:], rps[:, F : F + 1])
                p0 = 32 * (g % 4)
                nc.vector.tensor_scalar_mul(
                    r_sb[g // 4][p0 : p0 + 32, 0:F], rps[:, 0:F], dinv[:]
                )

            def _emit_r(g, k, n4, ean, rps):
                for i4 in range(n4):
                    i = 4 * k + i4
                    t = g * T_pad + i
                    nc.tensor.matmul(
                        rps[:],
                        lhsT=ean[:, 32 * i4 : 32 * i4 + 32],
                        rhs=xnm_sb[:, t * FW : t * FW + F + 1],
                        start=(i == 0),
                        stop=(i == T_pad - 1),
                    )

            for s in range(nsteps):
                # ---- LSTM cell (seg-major halves) ----
                if s == 0:
                    chunks = list(zip(qsE, [128, 128, 128, 17])) + [(Q1, 128), (Q2, K2)]
                    wts = wE
                else:
                    chunks = [(Q1, 128), (Q2, K2), (R1, 128), (R2, 73)]
                    wts = wD
                for half in range(2):
                    co = 128 * half
                    acts = {}
                    for part in range(2):
                        ps = psG.tile([128, 400], f32, tag="gates")
                        nch = len(chunks)
                        for ci, (ctile, kdim) in enumerate(chunks):
                            nc.tensor.matmul(
                                ps[:],
                                lhsT=ctile[0:kdim, co : co + 128],
                                rhs=wts[ci][0:kdim, 400 * part : 400 * part + 400],
                                start=(ci == 0),
                                stop=(ci == nch - 1),
                            )
                        if part == 0:
                            si = sbt.tile([128, F], f32, tag="si")
                            nc.scalar.activation(si[:], ps[:, 0:F], AF.Sigmoid)
                            sf = sbt.tile([128, F], f32, tag="sf")
                            nc.scalar.activation(sf[:], ps[:, F:400], AF.Sigmoid)
                            acts["i"], acts["f"] = si, sf
                        else:
                            tg = sbt.tile([128, F], f32, tag="tg")
                            nc.scalar.activation(tg[:], ps[:, 0:F], AF.Tanh)
                            so = sbt.tile([128, F], f32, tag="so")
                            nc.scalar.activation(so[:], ps[:, F:400], AF.Sigmoid)
                            acts["g"], acts["o"] = tg, so
                    ch = c_sb[half]
                    tmp = sbt.tile([128, F], f32, tag="tmp")
                    nc.vector.tensor_mul(tmp[:], acts["f"][:], ch[:])
                    nc.vector.tensor_mul(ch[:], acts["i"][:], acts["g"][:])
                    nc.vector.tensor_add(ch[:], tmp[:], ch[:])
                    tct = sbt.tile([128, F], f32, tag="tct")
                    nc.scalar.activation(tct[:], ch[:], AF.Tanh)
                    nc.vector.tensor_mul(h_sb[half][:], acts["o"][:], tct[:])

                # ---- q = h feat-major for attention + next LSTM input ----
                emit_hT(h_sb, Q1, Q2)

                # ---- attention, software-pipelined by one block ----
                ea_prev = None
                for g in range(BLOCKS):
                    ea = emit_e(g)
                    if ea_prev is not None:
                        emit_attn_tail(g - 1, ea_prev)
                    ea_prev = ea
                emit_attn_tail(BLOCKS - 1, ea_prev)

                if s < nsteps - 1:
                    emit_hT(r_sb, R1, R2)

            # ---------------- output: q_star = [h | r] ----------------
            for half in range(2):
                ro = 128 * half
                nc.sync.dma_start(qout_d[ro : ro + 128, 0:F], h_sb[half][:].bitcast(f32))
                if nsteps > 0:
                    nc.sync.dma_start(qout_d[ro : ro + 128, F : 2 * F], r_sb[half][:].bitcast(f32))

    nc.compile()
    return nc


def _get_program(T_pad: int) -> bass.Bass:
    nsteps = int(os.environ.get("KERNEL_NSTEPS", str(STEPS)))
    key = (T_pad, nsteps)
    if key not in _PROG_CACHE:
        _PROG_CACHE[key] = _build_program(T_pad, nsteps)
    return _PROG_CACHE[key]


def make_in_maps(x, batch, cos_coef, q_star, W_ih, W_hh, b_ih, b_hh):
    """Host-side shard + re-layout. Returns (in_maps, T_pad)."""
    x = np.ascontiguousarray(np.asarray(x, dtype=np.float32))
    batch = np.asarray(batch).astype(np.int64)
    cos = np.asarray(cos_coef, dtype=np.float32)
    qs = np.asarray(q_star, dtype=np.float32)
    W_ih = np.asarray(W_ih, dtype=np.float32)
    W_hh = np.asarray(W_hh, dtype=np.float32)
    bsum = (np.asarray(b_ih, dtype=np.float32) + np.asarray(b_hh, dtype=np.float32))

    counts = np.bincount(batch, minlength=B)
    starts = np.zeros(B + 1, dtype=np.int64)
    starts[1:] = np.cumsum(counts)
    blk_counts = counts.reshape(-1, BS).sum(axis=1)
    T_pad = int(max(1, -(-blk_counts.max() // 128)))
    NT = BLOCKS * T_pad
    BW = T_pad * 128

    bf = ml_dtypes.bfloat16

    # LSTM weight stacks (fp16)
    W_ihT = W_ih.T  # [400, 800]
    W_hhT = W_hh.T  # [200, 800]
    w0 = np.concatenate(
        [W_ihT, bsum[None, :], W_hhT, np.zeros((BS + 1, 800), np.float32)], axis=0
    ).astype(np.float16)  # [634, 800]; rows 529.. = W_hhT[128:200] + aug zeros
    WcT = W_ihT[:F] + W_hhT          # [200, 800]
    WrT = W_ihT[F:]                  # [200, 800]
    wc = np.concatenate(
        [WcT[0:128], WcT[128:200], np.zeros((BS + 1, 800), np.float32),
         WrT[0:128], WrT[128:200], bsum[None, :]], axis=0
    ).astype(np.float16)             # [434, 800]

    qc2c = np.zeros((BS + 1, 256), np.float16)
    qc2c[0:BS] = np.tile(100.0 * np.eye(BS, dtype=np.float32), (1, BLOCKS))
    qc2c[BS] = -100.0

    in_maps = []
    for c in range(CORES):
        seg0 = c * SEG_PER_CORE
        xnm = np.zeros((128, NT * FW), dtype=bf)
        xf = np.zeros((KAUG, NT * 128), dtype=np.float16)
        cwt = np.zeros((128, NT * BS), dtype=np.float16)
        xp = np.zeros((128, NT * F), dtype=np.float16)
        for g in range(BLOCKS):
            sa = seg0 + g * BS
            n0, n1 = int(starts[sa]), int(starts[sa + BS])
            cnt = n1 - n0
            js = (batch[n0:n1] - sa).astype(np.int64)

            xb = np.zeros((BW, FW), dtype=np.float32)
            xb[:cnt, :F] = x[n0:n1]
            xb[:cnt, F] = 1.0
            xnm[:, g * T_pad * FW : (g + 1) * T_pad * FW] = (
                xb.reshape(T_pad, 128, FW).transpose(1, 0, 2).reshape(128, T_pad * FW)
            ).astype(bf)
            xp[:, g * T_pad * F : (g + 1) * T_pad * F] = (
                xb[:, :F].reshape(T_pad, 128, F).transpose(1, 0, 2).reshape(128, T_pad * F)
            ).astype(np.float16)

            xfb = np.zeros((KAUG, BW), dtype=np.float32)
            xfb[0:F, :cnt] = x[n0:n1].T
            xfb[F + js, np.arange(cnt)] = 1.0
            xfb[F + BS, :] = 1.0
            xf[:, g * BW : (g + 1) * BW] = xfb.astype(np.float16)

            wb = np.zeros((BW, BS), dtype=np.float32)
            wb[np.arange(cnt), js] = cos[n0:n1]
            cwt[:, g * T_pad * BS : (g + 1) * T_pad * BS] = (
                wb.reshape(T_pad, 128, BS).transpose(1, 0, 2).reshape(128, T_pad * BS)
            ).astype(np.float16)

        qs0t = np.ones((401, 256), dtype=np.float16)
        qs0t[0:400] = qs[seg0 : seg0 + SEG_PER_CORE].T.astype(np.float16)
        in_maps.append(
            {
                "xnm": xnm,
                "xf1": np.ascontiguousarray(xf[0:128]),
                "xf2": np.ascontiguousarray(xf[128:KAUG]),
                "cwt": cwt,
                "xp": xp,
                "qs0t": qs0t,
                "w0": w0,
                "wc": wc,
                "qc2c": qc2c,
                "onesr": np.ones((1, 256), np.float16),
                "idf": np.eye(128, dtype=np.float32),
                "idb": np.eye(BS, dtype=np.float32).astype(bf),
            }
        )
    return in_maps, T_pad


def kernel(x, batch, cos_coef, q_star, W_ih, W_hh, b_ih, b_hh):
    global LAST_RESULT
    in_maps, T_pad = make_in_maps(
        x, batch, cos_coef, q_star, W_ih, W_hh, b_ih, b_hh
    )
    nc = _get_program(T_pad)
    res = run_bass_kernel_spmd(nc, in_maps, list(range(CORES)), trace=TRACE)
    LAST_RESULT = res
    out = np.zeros((B, 2 * F), dtype=np.float32)
    for c in range(CORES):
        out[c * SEG_PER_CORE : (c + 1) * SEG_PER_CORE] = res.results[c]["qout"]
    return out


# revision 11
# speedup vs baseline: 2.1939x; 1.2303x over previous
"""Trainium2 Bass kernel for CognitionNetwork (GNN message passing + LSTM attention).

Contract: kernel(**inputs) takes FULL inputs, returns FULL [2048, 400] q_star.
Shards 2048 conversations contiguously across 8 NeuronCores (256 segments each);
each block of 32 segments owns T_pad 128-node tiles (host re-layout).

v2 design (vs v0 per-tile gather):
  - attention scores e come from block-level matmuls contracting FEATURES:
    weights = per-block Q^T (reused across the block's tiles), rhs = a
    feature-major fp16 copy of x. The segment mask is folded into 33 extra
    "features" (indicator rows * 100 on both sides, ones row * -100), so
    e_aug = e + 100*onehot - 100 and exp(e_aug) is already the masked,
    unnormalized attention weight (off-segment entries underflow to 0).
  - exp runs on the scalar engine straight out of PSUM into a bf16 tile;
    per-tile PE transposes flip it node-major; the r matmul streams a bf16
    node-major x copy (ones column appended -> denominator for free).
  - all matmul operands are 16-bit (fp16 for e/LSTM, bf16 for r/phase0):
    1 cycle/row at any output width; fp32 masters kept for h/c/r state.
"""

import os
from contextlib import ExitStack

import ml_dtypes
import numpy as np

import concourse.bass as bass
import concourse.bacc as bacc
import concourse.tile as tile
from concourse import mybir
from concourse.bass_utils import run_bass_kernel_spmd

CORES = 8
B = 2048
F = 200
FW = 202              # node-major x tile width: 200 feats + ones col + pad
SEG_PER_CORE = B // CORES   # 256
BS = 32               # segments per block
BLOCKS = SEG_PER_CORE // BS  # 8
STEPS = 3
KAUG = F + BS + 1     # 233 feature rows incl mask aug
K2 = KAUG - 128       # 105 rows in chunk 2

TRACE = bool(int(os.environ.get("KERNEL_TRACE", "0")))
LAST_RESULT = None
_PROG_CACHE = {}


def _build_program(T_pad: int, nsteps: int = STEPS) -> bass.Bass:
    NT = BLOCKS * T_pad          # node tiles per core
    XFW = NT * 128               # feature-major x width (nodes)
    BW = T_pad * 128             # nodes per block

    nc = bacc.Bacc("TRN2", target_bir_lowering=False, debug=False)
    f32 = mybir.dt.float32
    f32r = mybir.dt.float32r
    f16 = mybir.dt.float16
    bf16 = mybir.dt.bfloat16
    AF = mybir.ActivationFunctionType

    xnm_d = nc.dram_tensor("xnm", [128, NT * FW], bf16, kind="ExternalInput").ap()
    xf1_d = nc.dram_tensor("xf1", [128, XFW], f16, kind="ExternalInput").ap()
    xf2_d = nc.dram_tensor("xf2", [K2, XFW], f16, kind="ExternalInput").ap()
    cwt_d = nc.dram_tensor("cwt", [128, NT * BS], f16, kind="ExternalInput").ap()
    xp_d = nc.dram_tensor("xp", [128, NT * F], f16, kind="ExternalInput").ap()
    qs0t_d = nc.dram_tensor("qs0t", [401, 256], f16, kind="ExternalInput").ap()
    w0_d = nc.dram_tensor("w0", [634, 800], f16, kind="ExternalInput").ap()
    wc_d = nc.dram_tensor("wc", [434, 800], f16, kind="ExternalInput").ap()
    qc2c_d = nc.dram_tensor("qc2c", [BS + 1, 256], f16, kind="ExternalInput").ap()
    ones_d = nc.dram_tensor("onesr", [1, 256], f16, kind="ExternalInput").ap()
    idf_d = nc.dram_tensor("idf", [128, 128], f32r, kind="ExternalInput").ap()
    idb_d = nc.dram_tensor("idb", [128, 128], bf16, kind="ExternalInput").ap()
    qout_d = nc.dram_tensor("qout", [256, 400], f32, kind="ExternalOutput").ap()

    with tile.TileContext(nc) as tc:
        with ExitStack() as ctx:
            res = ctx.enter_context(tc.tile_pool(name="res", bufs=1))
            state = ctx.enter_context(tc.tile_pool(name="state", bufs=1))
            eap = ctx.enter_context(tc.tile_pool(name="eap", bufs=2))
            xpp = ctx.enter_context(tc.tile_pool(name="xpp", bufs=3))
            eanp = ctx.enter_context(tc.tile_pool(name="eanp", bufs=2))
            sbt = ctx.enter_context(tc.tile_pool(name="sbt", bufs=2))
            psE = ctx.enter_context(tc.tile_pool(name="psE", bufs=2, space="PSUM"))
            psG = ctx.enter_context(tc.tile_pool(name="psG", bufs=2, space="PSUM"))
            psT = ctx.enter_context(tc.tile_pool(name="psT", bufs=2, space="PSUM"))
            psR = ctx.enter_context(tc.tile_pool(name="psR", bufs=2, space="PSUM"))

            # ---------------- resident loads ----------------
            idf = res.tile([128, 128], f32r)
            nc.sync.dma_start(idf[:], idf_d[:])
            idb = res.tile([128, 128], bf16)
            nc.sync.dma_start(idb[:], idb_d[:])

            cwt_sb = res.tile([128, NT * BS], f16)
            nc.sync.dma_start(cwt_sb[:], cwt_d[:])
            xnm_sb = res.tile([128, NT * FW], bf16)
            xf1_sb = res.tile([128, XFW], f16)
            xf2_sb = res.tile([K2, XFW], f16)

            # LSTM weights: step0 chunks E0..E3,F0,F1 ; steps>=1 chunks D0..D3
            wE = []
            for k, o in zip([128, 128, 128, 17, 128, K2], [0, 128, 256, 384, 401, 529]):
                t = res.tile([k, 800], f16, tag=f"wE{o}", name=f"wE{o}")
                nc.sync.dma_start(t[:], w0_d[o : o + k, :])
                wE.append(t)
            wD = []
            for k, o in zip([128, K2, 128, 73], [0, 128, 233, 361]):
                t = res.tile([k, 800], f16, tag=f"wD{o}", name=f"wD{o}")
                nc.sync.dma_start(t[:], wc_d[o : o + k, :])
                wD.append(t)

            # step-0 LSTM input chunks (q_star0^T from host)
            qsE = []
            for k, o in zip([128, 128, 128, 17], [0, 128, 256, 384]):
                t = res.tile([k, 256], f16, tag=f"qsE{o}", name=f"qsE{o}")
                nc.sync.dma_start(t[:], qs0t_d[o : o + k, :])
                qsE.append(t)

            # transposed-input chunks: Q1/Q2 (h^T + mask const), R1/R2 (r^T + ones)
            Q1 = res.tile([128, 256], f16, tag="Q1", name="Q1")
            Q2 = res.tile([K2, 256], f16, tag="Q2", name="Q2")
            nc.sync.dma_start(Q2[72:K2, :], qc2c_d[:])
            R1 = res.tile([128, 256], f16, tag="R1", name="R1")
            R2 = res.tile([73, 256], f16, tag="R2", name="R2")
            nc.sync.dma_start(R2[72:73, :], ones_d[:])

            # fp32 state masters (seg-major, two 128-partition halves)
            h_sb = [state.tile([128, F], f32r, tag=f"h{i}", name=f"h{i}") for i in range(2)]
            c_sb = [state.tile([128, F], f32, tag=f"c{i}", name=f"c{i}") for i in range(2)]
            r_sb = [state.tile([128, F], f32r, tag=f"r{i}", name=f"r{i}") for i in range(2)]
            for i in range(2):
                nc.vector.memset(c_sb[i][:], 0.0)

            # ---------------- phase 0: h0 = segment_sum(cos * x) ----------------
            # quad-stacked: 4 blocks (one 128-seg half) share one PSUM tile
            for q in range(2):
                h0ps = psR.tile([128, F], f32, tag="rblk")
                for a in range(4):
                    g = 4 * q + a
                    xp = xpp.tile([128, T_pad * F], f16, tag="xp")
                    nc.sync.dma_start(xp[:], xp_d[:, g * T_pad * F : (g + 1) * T_pad * F])
                    for i in range(T_pad):
                        t = g * T_pad + i
                        nc.tensor.matmul(
                            h0ps[32 * a : 32 * a + 32, :],
                            lhsT=cwt_sb[:, t * BS : (t + 1) * BS],
                            rhs=xp[:, i * F : (i + 1) * F],
                            start=(i == 0),
                            stop=(i == T_pad - 1),
                            tile_position=(0, 32 * a),
                        )
                nc.vector.tensor_copy(h_sb[q][:], h0ps[:])

            # bulk x loads (emitted after phase0 so its stream wins the queues)
            for g in range(BLOCKS):
                nc.sync.dma_start(xf1_sb[:, g * BW : (g + 1) * BW], xf1_d[:, g * BW : (g + 1) * BW])
                nc.sync.dma_start(xf2_sb[:, g * BW : (g + 1) * BW], xf2_d[:, g * BW : (g + 1) * BW])
                nc.sync.dma_start(
                    xnm_sb[:, g * T_pad * FW : (g + 1) * T_pad * FW],
                    xnm_d[:, g * T_pad * FW : (g + 1) * T_pad * FW],
                )

            def emit_hT(src_halves, dst1, dst2):
                """transpose seg-major [128,200] f32 halves into fp16 feat-major
                chunks: dst1[:, co:co+128] rows 0..127, dst2[0:72, ...] rows 128..199."""
                for half in range(2):
                    src = src_halves[half]
                    co = 128 * half
                    t1 = psT.tile([128, 128], f32r, tag="tp")
                    nc.tensor.transpose(t1[:], src[:, 0:128], idf[:])
                    nc.vector.tensor_copy(dst1[:, co : co + 128], t1[:].bitcast(f32))
                    t2 = psT.tile([72, 128], f32r, tag="tp")
                    nc.tensor.transpose(t2[:], src[:, 128:200], idf[:])
                    nc.vector.tensor_copy(dst2[0:72, co : co + 128], t2[:].bitcast(f32))

            emit_hT(h_sb, Q1, Q2)

            # ---------------- steps ----------------
            NCH = (BW + 511) // 512  # 512-col e-matmul chunks per block

            def emit_e(q):
                """e_aug matmuls + exp for 4 stacked blocks -> EA [128, BW] bf16."""
                ea = eap.tile([128, BW], bf16, tag="ea", name=f"ea")
                for k in range(NCH):
                    c0 = k * 512
                    cw = min(512, BW - c0)
                    pe = psE.tile([128, 512], f32, tag="pe")
                    for a in range(4):
                        g = 4 * q + a
                        nc.tensor.matmul(
                            pe[32 * a : 32 * a + 32, 0:cw],
                            lhsT=Q1[:, BS * g : BS * (g + 1)],
                            rhs=xf1_sb[:, g * BW + c0 : g * BW + c0 + cw],
                            start=True,
                            stop=False,
                            tile_position=(0, 32 * a),
                        )
                        nc.tensor.matmul(
                            pe[32 * a : 32 * a + 32, 0:cw],
                            lhsT=Q2[0:K2, BS * g : BS * (g + 1)],
                            rhs=xf2_sb[0:K2, g * BW + c0 : g * BW + c0 + cw],
                            start=False,
                            stop=True,
                            tile_position=(0, 32 * a),
                        )
                    nc.scalar.activation(ea[:, c0 : c0 + cw], pe[:, 0:cw], AF.Exp)
                return ea

            def emit_attn_tail(q, ea):
                """transpose EA node-major (4 tiles/instr), r matmuls, normalize."""
                rps = psR.tile([128, F + 1], f32, tag="rblk")
                ean_prev = None
                for i in range(T_pad):
                    tp = psT.tile([128, 128], bf16, tag="tp")
                    nc.tensor.transpose(tp[:], ea[:, 128 * i : 128 * i + 128], idb[:])
                    ean = eanp.tile([128, 128], bf16, tag="ean")
                    nc.vector.tensor_copy(ean[:], tp[:])
                    if ean_prev is not None:
                        _emit_r(q, i - 1, ean_prev, rps)
                    ean_prev = ean
                _emit_r(q, T_pad - 1, ean_prev, rps)
                dinv = sbt.tile([128, 1], f32, tag="dinv")
                nc.vector.reciprocal(dinv[:], rps[:, F : F + 1])
                nc.vector.tensor_scalar_mul(r_sb[q][:], rps[:, 0:F], dinv[:])

            def _emit_r(q, i, ean, rps):
                for a in range(4):
                    t = (4 * q + a) * T_pad + i
                    nc.tensor.matmul(
                        rps[32 * a : 32 * a + 32, :],
                        lhsT=ean[:, 32 * a : 32 * a + 32],
                        rhs=xnm_sb[:, t * FW : t * FW + F + 1],
                        start=(i == 0),
                        stop=(i == T_pad - 1),
                        tile_position=(0, 32 * a),
                    )

            for s in range(nsteps):
                # ---- LSTM cell (seg-major halves) ----
                if s == 0:
                    chunks = list(zip(qsE, [128, 128, 128, 17])) + [(Q1, 128), (Q2, K2)]
                    wts = wE
                else:
                    chunks = [(Q1, 128), (Q2, K2), (R1, 128), (R2, 73)]
                    wts = wD
                for half in range(2):
                    co = 128 * half
                    acts = {}
                    for part in range(2):
                        ps = psG.tile([128, 400], f32, tag="gates")
                        nch = len(chunks)
                        for ci, (ctile, kdim) in enumerate(chunks):
                            nc.tensor.matmul(
                                ps[:],
                                lhsT=ctile[0:kdim, co : co + 128],
                                rhs=wts[ci][0:kdim, 400 * part : 400 * part + 400],
                                start=(ci == 0),
                                stop=(ci == nch - 1),
                            )
                        if part == 0:
                            si = sbt.tile([128, F], f32, tag="si")
                            nc.scalar.activation(si[:], ps[:, 0:F], AF.Sigmoid)
                            sf = sbt.tile([128, F], f32, tag="sf")
                            nc.scalar.activation(sf[:], ps[:, F:400], AF.Sigmoid)
                            acts["i"], acts["f"] = si, sf
                        else:
                            tg = sbt.tile([128, F], f32, tag="tg")
                            nc.scalar.activation(tg[:], ps[:, 0:F], AF.Tanh)
                            so = sbt.tile([128, F], f32, tag="so")
                            nc.scalar.activation(so[:], ps[:, F:400], AF.Sigmoid)
                            acts["g"], acts["o"] = tg, so
                    ch = c_sb[half]
                    tmp = sbt.tile([128, F], f32, tag="tmp")
                    nc.vector.tensor_mul(tmp[:], acts["f"][:], ch[:])
                    nc.vector.tensor_mul(ch[:], acts["i"][:], acts["g"][:])
                    nc.vector.tensor_add(ch[:], tmp[:], ch[:])
                    tct = sbt.tile([128, F], f32, tag="tct")
                    nc.scalar.activation(tct[:], ch[:], AF.Tanh)
                    nc.vector.tensor_mul(h_sb[half][:], acts["o"][:], tct[:])

                # ---- q = h feat-major for attention + next LSTM input ----
                emit_hT(h_sb, Q1, Q2)

                # ---- attention, software-pipelined by one quad (half) ----
                ea0 = emit_e(0)
                ea1 = emit_e(1)
                emit_attn_tail(0, ea0)
                emit_attn_tail(1, ea1)

                if s < nsteps - 1:
                    emit_hT(r_sb, R1, R2)

            # ---------------- output: q_star = [h | r] ----------------
            for half in range(2):
                ro = 128 * half
                nc.sync.dma_start(qout_d[ro : ro + 128, 0:F], h_sb[half][:].bitcast(f32))
                if nsteps > 0:
                    nc.sync.dma_start(qout_d[ro : ro + 128, F : 2 * F], r_sb[half][:].bitcast(f32))

    nc.compile()
    return nc


def _get_program(T_pad: int) -> bass.Bass:
    nsteps = int(os.environ.get("KERNEL_NSTEPS", str(STEPS)))
    key = (T_pad, nsteps)
    if key not in _PROG_CACHE:
        _PROG_CACHE[key] = _build_program(T_pad, nsteps)
    return _PROG_CACHE[key]


def make_in_maps(x, batch, cos_coef, q_star, W_ih, W_hh, b_ih, b_hh):
    """Host-side shard + re-layout. Returns (in_maps, T_pad)."""
    x = np.ascontiguousarray(np.asarray(x, dtype=np.float32))
    batch = np.asarray(batch).astype(np.int64)
    cos = np.asarray(cos_coef, dtype=np.float32)
    qs = np.asarray(q_star, dtype=np.float32)
    W_ih = np.asarray(W_ih, dtype=np.float32)
    W_hh = np.asarray(W_hh, dtype=np.float32)
    bsum = (np.asarray(b_ih, dtype=np.float32) + np.asarray(b_hh, dtype=np.float32))

    counts = np.bincount(batch, minlength=B)
    starts = np.zeros(B + 1, dtype=np.int64)
    starts[1:] = np.cumsum(counts)
    blk_counts = counts.reshape(-1, BS).sum(axis=1)
    T_pad = int(max(1, -(-blk_counts.max() // 128)))
    NT = BLOCKS * T_pad
    BW = T_pad * 128

    bf = ml_dtypes.bfloat16

    # LSTM weight stacks (fp16)
    W_ihT = W_ih.T  # [400, 800]
    W_hhT = W_hh.T  # [200, 800]
    w0 = np.concatenate(
        [W_ihT, bsum[None, :], W_hhT, np.zeros((BS + 1, 800), np.float32)], axis=0
    ).astype(np.float16)  # [634, 800]; rows 529.. = W_hhT[128:200] + aug zeros
    WcT = W_ihT[:F] + W_hhT          # [200, 800]
    WrT = W_ihT[F:]                  # [200, 800]
    wc = np.concatenate(
        [WcT[0:128], WcT[128:200], np.zeros((BS + 1, 800), np.float32),
         WrT[0:128], WrT[128:200], bsum[None, :]], axis=0
    ).astype(np.float16)             # [434, 800]

    qc2c = np.zeros((BS + 1, 256), np.float16)
    qc2c[0:BS] = np.tile(100.0 * np.eye(BS, dtype=np.float32), (1, BLOCKS))
    qc2c[BS] = -100.0

    in_maps = []
    for c in range(CORES):
        seg0 = c * SEG_PER_CORE
        xnm = np.zeros((128, NT * FW), dtype=bf)
        xf = np.zeros((KAUG, NT * 128), dtype=np.float16)
        cwt = np.zeros((128, NT * BS), dtype=np.float16)
        xp = np.zeros((128, NT * F), dtype=np.float16)
        for g in range(BLOCKS):
            sa = seg0 + g * BS
            n0, n1 = int(starts[sa]), int(starts[sa + BS])
            cnt = n1 - n0
            js = (batch[n0:n1] - sa).astype(np.int64)

            xb = np.zeros((BW, FW), dtype=np.float32)
            xb[:cnt, :F] = x[n0:n1]
            xb[:cnt, F] = 1.0
            xnm[:, g * T_pad * FW : (g + 1) * T_pad * FW] = (
                xb.reshape(T_pad, 128, FW).transpose(1, 0, 2).reshape(128, T_pad * FW)
            ).astype(bf)
            xp[:, g * T_pad * F : (g + 1) * T_pad * F] = (
                xb[:, :F].reshape(T_pad, 128, F).transpose(1, 0, 2).reshape(128, T_pad * F)
            ).astype(np.float16)

            xfb = np.zeros((KAUG, BW), dtype=np.float32)
            xfb[0:F, :cnt] = x[n0:n1].T
            xfb[F + js, np.arange(cnt)] = 1.0
            xfb[F + BS, :] = 1.0
            xf[:, g * BW : (g + 1) * BW] = xfb.astype(np.float16)

            wb = np.zeros((BW, BS), dtype=np.float32)
            wb[np.arange(cnt), js] = cos[n0:n1]
            cwt[:, g * T_pad * BS : (g + 1) * T_pad * BS] = (
                wb.reshape(T_pad, 128, BS).transpose(1, 0, 2).reshape(128, T_pad * BS)
            ).astype(np.float16)

        qs0t = np.ones((401, 256), dtype=np.float16)
        qs0t[0:400] = qs[seg0 : seg0 + SEG_PER_CORE].T.astype(np.float16)
        in_maps.append(
            {
                "xnm": xnm,
                "xf1": np.ascontiguousarray(xf[0:128]),
                "xf2": np.ascontiguousarray(xf[128:KAUG]),
                "cwt": cwt,
                "xp": xp,
                "qs0t": qs0t,
                "w0": w0,
                "wc": wc,
                "qc2c": qc2c,
                "onesr": np.ones((1, 256), np.float16),
                "idf": np.eye(128, dtype=np.float32),
                "idb": np.eye(128, dtype=np.float32).astype(bf),
            }
        )
    return in_maps, T_pad


def kernel(x, batch, cos_coef, q_star, W_ih, W_hh, b_ih, b_hh):
    global LAST_RESULT
    in_maps, T_pad = make_in_maps(
        x, batch, cos_coef, q_star, W_ih, W_hh, b_ih, b_hh
    )
    nc = _get_program(T_pad)
    res = run_bass_kernel_spmd(nc, in_maps, list(range(CORES)), trace=TRACE)
    LAST_RESULT = res
    out = np.zeros((B, 2 * F), dtype=np.float32)
    for c in range(CORES):
        out[c * SEG_PER_CORE : (c + 1) * SEG_PER_CORE] = res.results[c]["qout"]
    return out


# revision 12
# speedup vs baseline: 2.6117x; 1.1904x over previous
"""Trainium2 Bass kernel for CognitionNetwork (GNN message passing + LSTM attention).

Contract: kernel(**inputs) takes FULL inputs, returns FULL [2048, 400] q_star.
Shards 2048 conversations contiguously across 8 NeuronCores (256 segments each);
each block of 32 segments owns T_pad 128-node tiles (host re-layout).

v2 design (vs v0 per-tile gather):
  - attention scores e come from block-level matmuls contracting FEATURES:
    weights = per-block Q^T (reused across the block's tiles), rhs = a
    feature-major fp16 copy of x. The segment mask is folded into 33 extra
    "features" (indicator rows * 100 on both sides, ones row * -100), so
    e_aug = e + 100*onehot - 100 and exp(e_aug) is already the masked,
    unnormalized attention weight (off-segment entries underflow to 0).
  - exp runs on the scalar engine straight out of PSUM into a bf16 tile;
    per-tile PE transposes flip it node-major; the r matmul streams a bf16
    node-major x copy (ones column appended -> denominator for free).
  - all matmul operands are 16-bit (fp16 for e/LSTM, bf16 for r/phase0):
    1 cycle/row at any output width; fp32 masters kept for h/c/r state.
"""

import os
from contextlib import ExitStack

import ml_dtypes
import numpy as np

import concourse.bass as bass
import concourse.bacc as bacc
import concourse.tile as tile
from concourse import mybir
from concourse.bass_utils import run_bass_kernel_spmd

CORES = 8
B = 2048
F = 200
FW = 201              # node-major x tile width: 200 feats + ones col
SEG_PER_CORE = B // CORES   # 256
BS = 32               # segments per block
BLOCKS = SEG_PER_CORE // BS  # 8
STEPS = 3
KAUG = F + BS + 1     # 233 feature rows incl mask aug
K2 = KAUG - 128       # 105 rows in chunk 2

TRACE = bool(int(os.environ.get("KERNEL_TRACE", "0")))
LAST_RESULT = None
_PROG_CACHE = {}


def _build_program(T_pad: int, nsteps: int = STEPS) -> bass.Bass:
    NT = BLOCKS * T_pad          # node tiles per core
    XFW = NT * 128               # feature-major x width (nodes)
    BW = T_pad * 128             # nodes per block

    nc = bacc.Bacc("TRN2", target_bir_lowering=False, debug=False)
    f32 = mybir.dt.float32
    f32r = mybir.dt.float32r
    f16 = mybir.dt.float16
    bf16 = mybir.dt.bfloat16
    AF = mybir.ActivationFunctionType

    xf1_d = nc.dram_tensor("xf1", [128, XFW], f16, kind="ExternalInput").ap()
    xf2_d = nc.dram_tensor("xf2", [K2, XFW], f16, kind="ExternalInput").ap()
    cwt_d = nc.dram_tensor("cwt", [128, NT * BS], f16, kind="ExternalInput").ap()
    xp_d = nc.dram_tensor("xp", [128, NT * FW], f16, kind="ExternalInput").ap()
    qs0t_d = nc.dram_tensor("qs0t", [401, 256], f16, kind="ExternalInput").ap()
    w0_d = nc.dram_tensor("w0", [634, 800], f16, kind="ExternalInput").ap()
    wc_d = nc.dram_tensor("wc", [434, 800], f16, kind="ExternalInput").ap()
    qc2c_d = nc.dram_tensor("qc2c", [BS + 1, 256], f16, kind="ExternalInput").ap()
    ones_d = nc.dram_tensor("onesr", [1, 256], f16, kind="ExternalInput").ap()
    idf_d = nc.dram_tensor("idf", [128, 128], f32r, kind="ExternalInput").ap()
    idb_d = nc.dram_tensor("idb", [128, 128], bf16, kind="ExternalInput").ap()
    qout_d = nc.dram_tensor("qout", [256, 400], f32, kind="ExternalOutput").ap()

    with tile.TileContext(nc) as tc:
        with ExitStack() as ctx:
            res = ctx.enter_context(tc.tile_pool(name="res", bufs=1))
            state = ctx.enter_context(tc.tile_pool(name="state", bufs=1))
            eap = ctx.enter_context(tc.tile_pool(name="eap", bufs=2))
            xpp = ctx.enter_context(tc.tile_pool(name="xpp", bufs=3))
            eanp = ctx.enter_context(tc.tile_pool(name="eanp", bufs=2))
            sbt = ctx.enter_context(tc.tile_pool(name="sbt", bufs=2))
            psE = ctx.enter_context(tc.tile_pool(name="psE", bufs=2, space="PSUM"))
            psG = ctx.enter_context(tc.tile_pool(name="psG", bufs=2, space="PSUM"))
            psT = ctx.enter_context(tc.tile_pool(name="psT", bufs=2, space="PSUM"))
            psR = ctx.enter_context(tc.tile_pool(name="psR", bufs=2, space="PSUM"))

            # ---------------- resident loads ----------------
            idf = res.tile([128, 128], f32r)
            nc.sync.dma_start(idf[:], idf_d[:])
            idb = res.tile([128, 128], bf16)
            nc.sync.dma_start(idb[:], idb_d[:])

            cwt_sb = res.tile([128, NT * BS], f16)
            xnm_sb = res.tile([128, NT * FW], bf16)
            xf1_sb = res.tile([128, XFW], f16)
            xf2_sb = res.tile([K2, XFW], f16)

            # transposed-input chunks: Q1/Q2 (h^T + mask const), R1/R2 (r^T + ones)
            Q1 = res.tile([128, 256], f16, tag="Q1", name="Q1")
            Q2 = res.tile([K2, 256], f16, tag="Q2", name="Q2")
            nc.sync.dma_start(Q2[72:K2, :], qc2c_d[:])
            R1 = res.tile([128, 256], f16, tag="R1", name="R1")
            R2 = res.tile([73, 256], f16, tag="R2", name="R2")
            nc.sync.dma_start(R2[72:73, :], ones_d[:])

            # fp32 state masters (seg-major, two 128-partition halves)
            h_sb = [state.tile([128, F], f32r, tag=f"h{i}", name=f"h{i}") for i in range(2)]
            c_sb = [state.tile([128, F], f32, tag=f"c{i}", name=f"c{i}") for i in range(2)]
            r_sb = [state.tile([128, F], f32r, tag=f"r{i}", name=f"r{i}") for i in range(2)]
            for i in range(2):
                nc.vector.memset(c_sb[i][:], 0.0)

            # ---------------- phase 0: h0 = segment_sum(cos * x) ----------------
            # quad-stacked; streams fp16 x (with ones col) per block, casting it
            # into the resident bf16 node-major copy as it goes
            for q in range(2):
                h0ps = psR.tile([128, F], f32, tag="rblk")
                for a in range(4):
                    g = 4 * q + a
                    nc.sync.dma_start(
                        cwt_sb[:, g * T_pad * BS : (g + 1) * T_pad * BS],
                        cwt_d[:, g * T_pad * BS : (g + 1) * T_pad * BS],
                    )
                    xpt = xpp.tile([128, T_pad * FW], f16, tag="xp")
                    nc.sync.dma_start(xpt[:], xp_d[:, g * T_pad * FW : (g + 1) * T_pad * FW])
                    for i in range(T_pad):
                        t = g * T_pad + i
                        nc.tensor.matmul(
                            h0ps[32 * a : 32 * a + 32, :],
                            lhsT=cwt_sb[:, t * BS : (t + 1) * BS],
                            rhs=xpt[:, i * FW : i * FW + F],
                            start=(i == 0),
                            stop=(i == T_pad - 1),
                            tile_position=(0, 32 * a),
                        )
                    nc.vector.tensor_copy(
                        xnm_sb[:, g * T_pad * FW : (g + 1) * T_pad * FW], xpt[:]
                    )
                nc.vector.tensor_copy(h_sb[q][:], h0ps[:])
            # bulk x loads (emitted after phase0 so its stream wins the queues)
            # LSTM weights: step0 chunks E0..E3,F0,F1 ; steps>=1 chunks D0..D3
            wE = []
            for k, o in zip([128, 128, 128, 17, 128, K2], [0, 128, 256, 384, 401, 529]):
                t = res.tile([k, 800], f16, tag=f"wE{o}", name=f"wE{o}")
                nc.sync.dma_start(t[:], w0_d[o : o + k, :])
                wE.append(t)
            wD = []
            for k, o in zip([128, K2, 128, 73], [0, 128, 233, 361]):
                t = res.tile([k, 800], f16, tag=f"wD{o}", name=f"wD{o}")
                nc.sync.dma_start(t[:], wc_d[o : o + k, :])
                wD.append(t)

            # step-0 LSTM input chunks (q_star0^T from host)
            qsE = []
            for k, o in zip([128, 128, 128, 17], [0, 128, 256, 384]):
                t = res.tile([k, 256], f16, tag=f"qsE{o}", name=f"qsE{o}")
                nc.sync.dma_start(t[:], qs0t_d[o : o + k, :])
                qsE.append(t)

            for g in range(BLOCKS):
                nc.sync.dma_start(xf1_sb[:, g * BW : (g + 1) * BW], xf1_d[:, g * BW : (g + 1) * BW])
                nc.sync.dma_start(xf2_sb[:, g * BW : (g + 1) * BW], xf2_d[:, g * BW : (g + 1) * BW])

            def emit_hT(src_halves, dst1, dst2, halves=(0, 1)):
                """transpose seg-major [128,200] f32r halves into fp16 feat-major
                chunks: dst1[:, co:co+128] rows 0..127, dst2[0:72, ...] rows 128..199."""
                for half in halves:
                    src = src_halves[half]
                    co = 128 * half
                    t1 = psT.tile([128, 128], f32r, tag="tp")
                    nc.tensor.transpose(t1[:], src[:, 0:128], idf[:])
                    nc.vector.tensor_copy(dst1[:, co : co + 128], t1[:].bitcast(f32))
                    t2 = psT.tile([72, 128], f32r, tag="tp")
                    nc.tensor.transpose(t2[:], src[:, 128:200], idf[:])
                    nc.vector.tensor_copy(dst2[0:72, co : co + 128], t2[:].bitcast(f32))

            emit_hT(h_sb, Q1, Q2)

            # ---------------- steps ----------------
            NCH = (BW + 511) // 512  # 512-col e-matmul chunks per block

            def emit_e(q):
                """e_aug matmuls + exp for 4 stacked blocks -> EA [128, BW] bf16."""
                ea = eap.tile([128, BW], bf16, tag="ea", name=f"ea")
                for k in range(NCH):
                    c0 = k * 512
                    cw = min(512, BW - c0)
                    pe = psE.tile([128, 512], f32, tag="pe")
                    for a in range(4):
                        g = 4 * q + a
                        nc.tensor.matmul(
                            pe[32 * a : 32 * a + 32, 0:cw],
                            lhsT=Q1[:, BS * g : BS * (g + 1)],
                            rhs=xf1_sb[:, g * BW + c0 : g * BW + c0 + cw],
                            start=True,
                            stop=False,
                            tile_position=(0, 32 * a),
                        )
                        nc.tensor.matmul(
                            pe[32 * a : 32 * a + 32, 0:cw],
                            lhsT=Q2[0:K2, BS * g : BS * (g + 1)],
                            rhs=xf2_sb[0:K2, g * BW + c0 : g * BW + c0 + cw],
                            start=False,
                            stop=True,
                            tile_position=(0, 32 * a),
                        )
                    nc.scalar.activation(ea[:, c0 : c0 + cw], pe[:, 0:cw], AF.Exp)
                return ea

            def emit_attn_tail(q, ea):
                """transpose EA node-major (4 tiles/instr), r matmuls, normalize."""
                rps = psR.tile([128, F + 1], f32, tag="rblk")
                ean_prev = None
                for i in range(T_pad):
                    tp = psT.tile([128, 128], bf16, tag="tp")
                    nc.tensor.transpose(tp[:], ea[:, 128 * i : 128 * i + 128], idb[:])
                    ean = eanp.tile([128, 128], bf16, tag="ean")
                    nc.vector.tensor_copy(ean[:], tp[:])
                    if ean_prev is not None:
                        _emit_r(q, i - 1, ean_prev, rps)
                    ean_prev = ean
                _emit_r(q, T_pad - 1, ean_prev, rps)
                dinv = sbt.tile([128, 1], f32, tag="dinv")
                nc.vector.reciprocal(dinv[:], rps[:, F : F + 1])
                nc.vector.tensor_scalar_mul(r_sb[q][:], rps[:, 0:F], dinv[:])

            def _emit_r(q, i, ean, rps):
                for a in range(4):
                    t = (4 * q + a) * T_pad + i
                    nc.tensor.matmul(
                        rps[32 * a : 32 * a + 32, :],
                        lhsT=ean[:, 32 * a : 32 * a + 32],
                        rhs=xnm_sb[:, t * FW : t * FW + F + 1],
                        start=(i == 0),
                        stop=(i == T_pad - 1),
                        tile_position=(0, 32 * a),
                    )

            for s in range(nsteps):
                # ---- LSTM cell (seg-major halves) ----
                if s == 0:
                    chunks = list(zip(qsE, [128, 128, 128, 17])) + [(Q1, 128), (Q2, K2)]
                    wts = wE
                else:
                    chunks = [(Q1, 128), (Q2, K2), (R1, 128), (R2, 73)]
                    wts = wD
                def lstm_half(half):
                    co = 128 * half
                    acts = {}
                    for part in range(2):
                        ps = psG.tile([128, 400], f32, tag="gates")
                        nch = len(chunks)
                        for ci, (ctile, kdim) in enumerate(chunks):
                            nc.tensor.matmul(
                                ps[:],
                                lhsT=ctile[0:kdim, co : co + 128],
                                rhs=wts[ci][0:kdim, 400 * part : 400 * part + 400],
                                start=(ci == 0),
                                stop=(ci == nch - 1),
                            )
                        if part == 0:
                            si = sbt.tile([128, F], f32, tag="si")
                            nc.scalar.activation(si[:], ps[:, 0:F], AF.Sigmoid)
                            sf = sbt.tile([128, F], f32, tag="sf")
                            nc.scalar.activation(sf[:], ps[:, F:400], AF.Sigmoid)
                            acts["i"], acts["f"] = si, sf
                        else:
                            tg = sbt.tile([128, F], f32, tag="tg")
                            nc.scalar.activation(tg[:], ps[:, 0:F], AF.Tanh)
                            so = sbt.tile([128, F], f32, tag="so")
                            nc.scalar.activation(so[:], ps[:, F:400], AF.Sigmoid)
                            acts["g"], acts["o"] = tg, so
                    ch = c_sb[half]
                    tmp = sbt.tile([128, F], f32, tag="tmp")
                    nc.vector.tensor_mul(tmp[:], acts["f"][:], ch[:])
                    nc.vector.tensor_mul(ch[:], acts["i"][:], acts["g"][:])
                    nc.vector.tensor_add(ch[:], tmp[:], ch[:])
                    tct = sbt.tile([128, F], f32, tag="tct")
                    nc.scalar.activation(tct[:], ch[:], AF.Tanh)
                    nc.vector.tensor_mul(h_sb[half][:], acts["o"][:], tct[:])

                lstm_half(0)
                lstm_half(1)

                # ---- per-half h^T then e-matmuls: attention starts while the
                # other half's LSTM tail still runs on scalar/vector ----
                emit_hT(h_sb, Q1, Q2, halves=(0,))
                ea0 = emit_e(0)
                emit_hT(h_sb, Q1, Q2, halves=(1,))
                ea1 = emit_e(1)
                emit_attn_tail(0, ea0)
                if s < nsteps - 1:
                    emit_hT(r_sb, R1, R2, halves=(0,))
                emit_attn_tail(1, ea1)
                if s < nsteps - 1:
                    emit_hT(r_sb, R1, R2, halves=(1,))

            # ---------------- output: q_star = [h | r] ----------------
            for half in range(2):
                ro = 128 * half
                nc.sync.dma_start(qout_d[ro : ro + 128, 0:F], h_sb[half][:].bitcast(f32))
                if nsteps > 0:
                    nc.sync.dma_start(qout_d[ro : ro + 128, F : 2 * F], r_sb[half][:].bitcast(f32))

    nc.compile()
    return nc


def _get_program(T_pad: int) -> bass.Bass:
    nsteps = int(os.environ.get("KERNEL_NSTEPS", str(STEPS)))
    key = (T_pad, nsteps)
    if key not in _PROG_CACHE:
        _PROG_CACHE[key] = _build_program(T_pad, nsteps)
    return _PROG_CACHE[key]


def make_in_maps(x, batch, cos_coef, q_star, W_ih, W_hh, b_ih, b_hh):
    """Host-side shard + re-layout. Returns (in_maps, T_pad)."""
    x = np.ascontiguousarray(np.asarray(x, dtype=np.float32))
    batch = np.asarray(batch).astype(np.int64)
    cos = np.asarray(cos_coef, dtype=np.float32)
    qs = np.asarray(q_star, dtype=np.float32)
    W_ih = np.asarray(W_ih, dtype=np.float32)
    W_hh = np.asarray(W_hh, dtype=np.float32)
    bsum = (np.asarray(b_ih, dtype=np.float32) + np.asarray(b_hh, dtype=np.float32))

    counts = np.bincount(batch, minlength=B)
    starts = np.zeros(B + 1, dtype=np.int64)
    starts[1:] = np.cumsum(counts)
    blk_counts = counts.reshape(-1, BS).sum(axis=1)
    T_pad = int(max(1, -(-blk_counts.max() // 128)))
    NT = BLOCKS * T_pad
    BW = T_pad * 128

    bf = ml_dtypes.bfloat16

    # LSTM weight stacks (fp16)
    W_ihT = W_ih.T  # [400, 800]
    W_hhT = W_hh.T  # [200, 800]
    w0 = np.concatenate(
        [W_ihT, bsum[None, :], W_hhT, np.zeros((BS + 1, 800), np.float32)], axis=0
    ).astype(np.float16)  # [634, 800]; rows 529.. = W_hhT[128:200] + aug zeros
    WcT = W_ihT[:F] + W_hhT          # [200, 800]
    WrT = W_ihT[F:]                  # [200, 800]
    wc = np.concatenate(
        [WcT[0:128], WcT[128:200], np.zeros((BS + 1, 800), np.float32),
         WrT[0:128], WrT[128:200], bsum[None, :]], axis=0
    ).astype(np.float16)             # [434, 800]

    qc2c = np.zeros((BS + 1, 256), np.float16)
    qc2c[0:BS] = np.tile(100.0 * np.eye(BS, dtype=np.float32), (1, BLOCKS))
    qc2c[BS] = -100.0

    in_maps = []
    for c in range(CORES):
        seg0 = c * SEG_PER_CORE
        xf = np.zeros((KAUG, NT * 128), dtype=np.float16)
        cwt = np.zeros((128, NT * BS), dtype=np.float16)
        xp = np.zeros((128, NT * FW), dtype=np.float16)
        for g in range(BLOCKS):
            sa = seg0 + g * BS
            n0, n1 = int(starts[sa]), int(starts[sa + BS])
            cnt = n1 - n0
            js = (batch[n0:n1] - sa).astype(np.int64)

            xb = np.zeros((BW, FW), dtype=np.float32)
            xb[:cnt, :F] = x[n0:n1]
            xb[:cnt, F] = 1.0
            xp[:, g * T_pad * FW : (g + 1) * T_pad * FW] = (
                xb.reshape(T_pad, 128, FW).transpose(1, 0, 2).reshape(128, T_pad * FW)
            ).astype(np.float16)

            xfb = np.zeros((KAUG, BW), dtype=np.float32)
            xfb[0:F, :cnt] = x[n0:n1].T
            xfb[F + js, np.arange(cnt)] = 1.0
            xfb[F + BS, :] = 1.0
            xf[:, g * BW : (g + 1) * BW] = xfb.astype(np.float16)

            wb = np.zeros((BW, BS), dtype=np.float32)
            wb[np.arange(cnt), js] = cos[n0:n1]
            cwt[:, g * T_pad * BS : (g + 1) * T_pad * BS] = (
                wb.reshape(T_pad, 128, BS).transpose(1, 0, 2).reshape(128, T_pad * BS)
            ).astype(np.float16)

        qs0t = np.ones((401, 256), dtype=np.float16)
        qs0t[0:400] = qs[seg0 : seg0 + SEG_PER_CORE].T.astype(np.float16)
        in_maps.append(
            {
                "xf1": np.ascontiguousarray(xf[0:128]),
                "xf2": np.ascontiguousarray(xf[128:KAUG]),
                "cwt": cwt,
                "xp": xp,
                "qs0t": qs0t,
                "w0": w0,
                "wc": wc,
                "qc2c": qc2c,
                "onesr": np.ones((1, 256), np.float16),
                "idf": np.eye(128, dtype=np.float32),
                "idb": np.eye(128, dtype=np.float32).astype(bf),
            }
        )
    return in_maps, T_pad


def kernel(x, batch, cos_coef, q_star, W_ih, W_hh, b_ih, b_hh):
    global LAST_RESULT
    in_maps, T_pad = make_in_maps(
        x, batch, cos_coef, q_star, W_ih, W_hh, b_ih, b_hh
    )
    nc = _get_program(T_pad)
    res = run_bass_kernel_spmd(nc, in_maps, list(range(CORES)), trace=TRACE)
    LAST_RESULT = res
    out = np.zeros((B, 2 * F), dtype=np.float32)
    for c in range(CORES):
        out[c * SEG_PER_CORE : (c + 1) * SEG_PER_CORE] = res.results[c]["qout"]
    return out
